# revision 19
# baseline (speedup 1.0000x reference)
"""AdaModConv1D on 8 TRN2 NeuronCores — pure data parallel (1 sample/core).

Math: s = softplus(ltnt @ Wd + bd) + 1          [B, C]
      d = rsqrt(einsum('kcf,bc->bf', K^2, s^2) + eps)
      y = conv1d(x * s, K, SAME) * d

Per-core trick: each core owns ONE sample, so the input modulation (x*s) and
output demodulation (y*d) fold into the conv weights:
      w''[k,c,f] = K[k,c,f] * s[c] * d[f];   y = conv1d(x, w'')
making the device inner loop a pure 3-tap conv1d = 3 accumulating matmuls
per output tile on the PE, with a tiny on-device prologue computing w''.

Layout: host pre-transposes each sample to channels-first bf16
[128 = (half, c), 32770] with one halo column on each side (half = L split in
two so both partition halves are used; the two 64x64 conv quadrants run at
PE tile_position (0,0) and (64,64) concurrently). Output comes back as
y^T [128 = (half, f), 32768] bf16 and is re-transposed on host.
"""

import os
import sys

sys.path.insert(0, "/opt/trn_rl_repo")

import numpy as np
import ml_dtypes

BF16 = ml_dtypes.bfloat16

B, L, C = 8, 65536, 64
F, KW, DL = 64, 3, 256
EPS = 1e-8
H = L // 2          # 32768, half length per partition group
NCHUNK = 4          # DMA chunks per direction
CHW = H // NCHUNK   # 4096 columns per chunk
NGRP = H // 512     # 64 matmul groups of 512 outputs x 2 halves
PARW = 256 + 2 + 1 + 192  # packed params width

_cached = {}


def _build():
    import concourse.bass as bass
    import concourse.bacc as bacc
    import concourse.mybir as mybir
    import concourse.tile as tile

    dt = mybir.dt
    nc = bacc.Bacc("TRN2", target_bir_lowering=False, debug=False, num_devices=8)

    xin = nc.declare_dram_parameter("xin", [128, H + 2], dt.bfloat16, isOutput=False)
    # host-packed params, one DMA: [wd_dup 256 | lt 2 | bd_dup 1 | ker 192]
    par = nc.declare_dram_parameter("par", [128, PARW], dt.bfloat16, isOutput=False)
    yout = nc.declare_dram_parameter("yout", [128, H], dt.bfloat16, isOutput=True)

    with tile.TileContext(nc) as tc:
        with (
            tc.tile_pool(name="xin", bufs=1) as xin_pool,
            tc.tile_pool(name="yout", bufs=1) as yout_pool,
            tc.tile_pool(name="pre", bufs=1) as pre,
            tc.tile_pool(name="pp", bufs=2, space="PSUM") as pre_psum,
            tc.tile_pool(name="cp", bufs=6, space="PSUM") as conv_psum,
        ):
            # ---- ACT table prewarm: dummy exp/ln/copy with no DMA deps so
            # all three LUT segments load during the DMA window, not inside
            # the s-chain critical path ----
            eps_sb = pre.tile([128, 1], dt.float32, tag="eps")
            nc.vector.memset(eps_sb[:], EPS)
            scr = pre.tile([128, 1], dt.float32, tag="scr")
            nc.scalar.activation(scr[:], eps_sb[:], mybir.ActivationFunctionType.Exp)
            nc.scalar.activation(scr[:], eps_sb[:], mybir.ActivationFunctionType.Ln)
            nc.scalar.activation(scr[:], eps_sb[:], mybir.ActivationFunctionType.Copy)

            # ---- packed param DMA: MUST be issued before the chunk DMAs —
            # same HWDGE FIFO lane, completion milestones are cumulative ----
            par_sb = pre.tile([128, PARW], dt.bfloat16, tag="par")
            nc.sync.dma_start(out=par_sb[:], in_=par[:])
            lt_sb = par_sb[:, 256:258]
            bd_sb = par_sb[:, 258:259]
            ker_flat = par_sb[:, 259 : 259 + KW * F]

            # ---- input chunk DMAs ----
            xc = []
            for c in range(NCHUNK):
                t = xin_pool.tile([128, CHW + 2], dt.bfloat16, tag=f"xin{c}")
                nc.sync.dma_start(out=t[:], in_=xin[:, c * CHW : c * CHW + CHW + 2])
                xc.append(t)

            # ---- prologue: s = softplus(ltnt @ Wd + bd) + 1, per (half, c) ----
            s_pre = pre_psum.tile([128, 1], dt.float32, tag="pp")
            for a in range(2):
                nc.tensor.matmul(
                    s_pre[:],
                    lhsT=par_sb[:, a * 128 : (a + 1) * 128],
                    rhs=lt_sb[:, a : a + 1],
                    start=(a == 0),
                    stop=(a == 1),
                )
            # softplus(p) = ln(1 + exp(p)); only exp/ln/copy exist in one ACT set
            e_sb = pre.tile([128, 1], dt.float32, tag="e")
            nc.scalar.activation(
                e_sb[:], s_pre[:], mybir.ActivationFunctionType.Exp, bias=bd_sb[:]
            )
            nc.vector.tensor_scalar_add(e_sb[:], e_sb[:], 1.0)
            s_sb = pre.tile([128, 1], dt.float32, tag="s")
            nc.scalar.activation(s_sb[:], e_sb[:], mybir.ActivationFunctionType.Ln)
            nc.vector.tensor_scalar_add(s_sb[:], s_sb[:], 1.0)
            s2_sb = pre.tile([128, 1], dt.bfloat16, tag="s2")
            nc.vector.tensor_mul(s2_sb[:], s_sb[:], s_sb[:])

            # ---- d = 1/sqrt(sum_kc K^2 s^2 + eps) as [1, F] ----
            k2_sb = pre.tile([128, KW * F], dt.bfloat16, tag="k2")
            nc.vector.tensor_mul(k2_sb[:], ker_flat[:], ker_flat[:])
            dpre = pre_psum.tile([1, F], dt.float32, tag="pp")
            for k in range(KW):
                nc.tensor.matmul(
                    dpre[:],
                    lhsT=s2_sb[0:64, :],
                    rhs=k2_sb[0:64, k * F : (k + 1) * F],
                    start=(k == 0),
                    stop=(k == KW - 1),
                )
            # rsqrt(v) = exp(-0.5 * ln(v)) — keeps ACT funcs within one LUT set
            lnv = pre.tile([1, F], dt.float32, tag="lnv")
            nc.scalar.activation(
                lnv[:], dpre[:], mybir.ActivationFunctionType.Ln, bias=eps_sb[0:1, :]
            )
            d_sb = pre.tile([1, F], dt.float32, tag="d")
            nc.scalar.activation(
                d_sb[:], lnv[:], mybir.ActivationFunctionType.Exp, scale=-0.5
            )
            d3_sb = pre.tile([1, KW * F], dt.bfloat16, tag="d3")
            for k in range(KW):
                nc.vector.tensor_copy(d3_sb[:, k * F : (k + 1) * F], d_sb[:])

            # ---- w''[(h,c),(k,f)] = K[k,c,f] * d[f] * s[c], bf16 ----
            ones = pre.tile([1, 64], dt.bfloat16, tag="ones")
            nc.vector.memset(ones[:], 1.0)
            dmat = pre_psum.tile([128, KW * F], dt.float32, tag="pp")
            for h in range(2):
                nc.tensor.matmul(
                    dmat[h * 64 : (h + 1) * 64, :],
                    lhsT=ones[:],
                    rhs=d3_sb[:],
                    start=True,
                    stop=True,
                )
            wtmp = pre.tile([128, KW * F], dt.float32, tag="wtmp")
            nc.vector.tensor_mul(wtmp[:], ker_flat[:], dmat[:])
            wfin = pre.tile([128, KW * F], dt.bfloat16, tag="wfin")
            nc.vector.tensor_scalar_mul(wfin[:], wtmp[:], s_sb[:])

            # ---- main conv loop: 4 concurrent 64x64 PE quadrants per tap,
            # two 512-col windows (W0->bank X normal layout, W1->bank Y with
            # partition halves swapped; the host unswizzles odd windows) ----
            yc = [
                yout_pool.tile(
                    [128, CHW], dt.bfloat16, name=f"yout{c}", tag=f"yout{c}"
                )
                for c in range(NCHUNK)
            ]
            PPC = NGRP // 2 // NCHUNK  # pairs per chunk
            for gp in range(NGRP // 2):
                c = gp // PPC
                j0 = (gp % PPC) * 2
                b0, b1 = j0 * 512, (j0 + 1) * 512
                psX = conv_psum.tile([128, 512], dt.float32, name="psX", tag="convps")
                psY = conv_psum.tile([128, 512], dt.float32, name="psY", tag="convps")
                x = xc[c]
                for k in range(KW):
                    st, sp = (k == 0), (k == KW - 1)
                    wA = wfin[0:64, k * F : (k + 1) * F]
                    wB = wfin[64:128, k * F : (k + 1) * F]
                    nc.tensor.matmul(
                        psX[0:64, :], lhsT=wA, rhs=x[0:64, b0 + k : b0 + k + 512],
                        start=st, stop=sp, skip_group_check=True,
                    )
                    nc.tensor.matmul(
                        psX[64:128, :], lhsT=wB, rhs=x[64:128, b0 + k : b0 + k + 512],
                        start=st, stop=sp, skip_group_check=True,
                    )
                    nc.tensor.matmul(
                        psY[64:128, :], lhsT=wA, rhs=x[0:64, b1 + k : b1 + k + 512],
                        start=st, stop=sp, skip_group_check=True,
                    )
                    nc.tensor.matmul(
                        psY[0:64, :], lhsT=wB, rhs=x[64:128, b1 + k : b1 + k + 512],
                        start=st, stop=sp, skip_group_check=True,
                    )
                for w, ps in ((0, psX), (1, psY)):
                    dst = yc[c][:, b0 + w * 512 : b0 + w * 512 + 512]
                    if (2 * gp + w) % 2 == 0:
                        nc.vector.tensor_copy(dst, ps[:])
                    else:
                        nc.scalar.copy(dst, ps[:])
                if gp % 4 == 3:
                    p0 = (j0 - 6) * 512
                    if gp == NGRP // 2 - 1:
                        # final piece split so the trailing transfer is short
                        for q in (0, 1024, 2048, 3072):
                            nc.sync.dma_start(
                                out=yout[:, c * CHW + p0 + q : c * CHW + p0 + q + 1024],
                                in_=yc[c][:, p0 + q : p0 + q + 1024],
                            )
                    else:
                        nc.sync.dma_start(
                            out=yout[:, c * CHW + p0 : c * CHW + p0 + 4096],
                            in_=yc[c][:, p0 : p0 + 4096],
                        )

    nc.compile()
    return nc


def _get_nc():
    if "nc" not in _cached:
        _cached["nc"] = _build()
    return _cached["nc"]


def pack_params(ltnt_b, kernel, Wd, bd):
    """[128, PARW] f32: wd dup'd per half (2 x [128,128]), lt, bd dup, ker."""
    par = np.empty((128, PARW), dtype=BF16)
    for a in range(2):
        par[:, a * 128 : a * 128 + 64] = Wd[a * 128 : (a + 1) * 128, :]
        par[:, a * 128 + 64 : (a + 1) * 128] = Wd[a * 128 : (a + 1) * 128, :]
    par[:, 256] = ltnt_b[0:128]
    par[:, 257] = ltnt_b[128:256]
    par[:, 258] = np.tile(bd, 2)
    # ker block: par[p, 259 + k*64 + f] = kernel[k, p % 64, f]
    kblk = kernel.transpose(1, 0, 2).reshape(64, KW * F)  # [c, (k,f)]
    par[:, 259:] = np.tile(kblk, (2, 1))
    return par


def make_xin(data_b):
    xt = data_b.reshape(2, H, C).transpose(0, 2, 1)  # [2, C, H]
    xin = np.zeros((128, H + 2), dtype=BF16)
    xin[:, 1 : H + 1] = xt.reshape(128, H).astype(BF16)
    xin[64:128, 0] = xt[0, :, -1].astype(BF16)  # x[H-1] left halo of half 1
    xin[0:64, H + 1] = xt[1, :, 0].astype(BF16)  # x[H] right halo of half 0
    return xin


def kernel(data, ltnt, kernel, Wd, bd):
    from concourse import bass_utils

    nc = _get_nc()

    data = np.asarray(data, dtype=np.float32)
    ltnt = np.asarray(ltnt, dtype=np.float32)
    kf = np.asarray(kernel, dtype=np.float32)
    wdf = np.asarray(Wd, dtype=np.float32)
    bdf = np.asarray(bd, dtype=np.float32)

    in_maps = [
        {"xin": make_xin(data[b]), "par": pack_params(ltnt[b], kf, wdf, bdf)}
        for b in range(B)
    ]

    try:
        res = bass_utils.run_bass_kernel_spmd(nc, in_maps, core_ids=list(range(B)))
    except Exception:
        # transient NRT_EXEC_UNIT_UNRECOVERABLE seen when the device was left
        # wedged by a prior process; one retry after a pause clears it
        import time

        time.sleep(15)
        res = bass_utils.run_bass_kernel_spmd(nc, in_maps, core_ids=list(range(B)))

    out = np.empty((B, L, C), dtype=np.float32)
    even = (np.arange(NGRP) % 2 == 0)[None, :, None]
    for b in range(B):
        yo = np.asarray(res.results[b]["yout"]).astype(np.float32)  # [128, H]
        yr = yo.reshape(2, F, NGRP, 512)  # [rowhalf, f, window, l]
        h0 = np.where(even, yr[0], yr[1])  # odd windows come halves-swapped
        h1 = np.where(even, yr[1], yr[0])
        out[b, :H] = h0.transpose(1, 2, 0).reshape(H, F)
        out[b, H:] = h1.transpose(1, 2, 0).reshape(H, F)
    return out


# revision 20
# speedup vs baseline: 1.0557x; 1.0557x over previous
"""AdaModConv1D on 8 TRN2 NeuronCores — pure data parallel (1 sample/core).

Math: s = softplus(ltnt @ Wd + bd) + 1          [B, C]
      d = rsqrt(einsum('kcf,bc->bf', K^2, s^2) + eps)
      y = conv1d(x * s, K, SAME) * d

Per-core trick: each core owns ONE sample, so the input modulation (x*s) and
output demodulation (y*d) fold into the conv weights:
      w''[k,c,f] = K[k,c,f] * s[c] * d[f];   y = conv1d(x, w'')
making the device inner loop a pure 3-tap conv1d = 3 accumulating matmuls
per output tile on the PE, with a tiny on-device prologue computing w''.

Layout: host pre-transposes each sample to channels-first bf16
[128 = (half, c), 32770] with one halo column on each side (half = L split in
two so both partition halves are used; the two 64x64 conv quadrants run at
PE tile_position (0,0) and (64,64) concurrently). Output comes back as
y^T [128 = (half, f), 32768] bf16 and is re-transposed on host.
"""

import os
import sys

sys.path.insert(0, "/opt/trn_rl_repo")

import numpy as np
import ml_dtypes

BF16 = ml_dtypes.bfloat16

B, L, C = 8, 65536, 64
F, KW, DL = 64, 3, 256
EPS = 1e-8
H = L // 2          # 32768, half length per partition group
NCHUNK = 8          # DMA chunks per direction
CHW = H // NCHUNK   # 4096 columns per chunk
NGRP = H // 512     # 64 matmul groups of 512 outputs x 2 halves
PARW = 256 + 2 + 1 + 192  # packed params width

_cached = {}


def _build():
    import concourse.bass as bass
    import concourse.bacc as bacc
    import concourse.mybir as mybir
    import concourse.tile as tile

    dt = mybir.dt
    nc = bacc.Bacc("TRN2", target_bir_lowering=False, debug=False, num_devices=8)

    xin = nc.declare_dram_parameter("xin", [128, H + 2], dt.bfloat16, isOutput=False)
    # host-packed params, one DMA: [wd_dup 256 | lt 2 | bd_dup 1 | ker 192]
    par = nc.declare_dram_parameter("par", [128, PARW], dt.bfloat16, isOutput=False)
    yout = nc.declare_dram_parameter("yout", [128, H], dt.bfloat16, isOutput=True)

    with tile.TileContext(nc) as tc:
        with (
            tc.tile_pool(name="xin", bufs=1) as xin_pool,
            tc.tile_pool(name="yout", bufs=1) as yout_pool,
            tc.tile_pool(name="pre", bufs=1) as pre,
            tc.tile_pool(name="pp", bufs=2, space="PSUM") as pre_psum,
            tc.tile_pool(name="cp", bufs=6, space="PSUM") as conv_psum,
        ):
            # ---- ACT table prewarm: dummy exp/ln/copy with no DMA deps so
            # all three LUT segments load during the DMA window, not inside
            # the s-chain critical path ----
            eps_sb = pre.tile([128, 1], dt.float32, tag="eps")
            nc.vector.memset(eps_sb[:], EPS)
            scr = pre.tile([128, 1], dt.float32, tag="scr")
            nc.scalar.activation(scr[:], eps_sb[:], mybir.ActivationFunctionType.Exp)
            nc.scalar.activation(scr[:], eps_sb[:], mybir.ActivationFunctionType.Ln)
            nc.scalar.activation(scr[:], eps_sb[:], mybir.ActivationFunctionType.Copy)

            # ---- packed param DMA: MUST be issued before the chunk DMAs —
            # same HWDGE FIFO lane, completion milestones are cumulative ----
            par_sb = pre.tile([128, PARW], dt.bfloat16, tag="par")
            nc.sync.dma_start(out=par_sb[:], in_=par[:])
            lt_sb = par_sb[:, 256:258]
            bd_sb = par_sb[:, 258:259]
            ker_flat = par_sb[:, 259 : 259 + KW * F]

            # ---- input chunk DMAs ----
            xc = []
            for c in range(NCHUNK):
                t = xin_pool.tile([128, CHW + 2], dt.bfloat16, tag=f"xin{c}")
                nc.sync.dma_start(out=t[:], in_=xin[:, c * CHW : c * CHW + CHW + 2])
                xc.append(t)

            # ---- prologue: s = softplus(ltnt @ Wd + bd) + 1, per (half, c) ----
            s_pre = pre_psum.tile([128, 1], dt.float32, tag="pp")
            for a in range(2):
                nc.tensor.matmul(
                    s_pre[:],
                    lhsT=par_sb[:, a * 128 : (a + 1) * 128],
                    rhs=lt_sb[:, a : a + 1],
                    start=(a == 0),
                    stop=(a == 1),
                )
            # softplus(p) = ln(1 + exp(p)); only exp/ln/copy exist in one ACT set
            e_sb = pre.tile([128, 1], dt.float32, tag="e")
            nc.scalar.activation(
                e_sb[:], s_pre[:], mybir.ActivationFunctionType.Exp, bias=bd_sb[:]
            )
            nc.vector.tensor_scalar_add(e_sb[:], e_sb[:], 1.0)
            s_sb = pre.tile([128, 1], dt.float32, tag="s")
            nc.scalar.activation(s_sb[:], e_sb[:], mybir.ActivationFunctionType.Ln)
            nc.vector.tensor_scalar_add(s_sb[:], s_sb[:], 1.0)
            s2_sb = pre.tile([128, 1], dt.bfloat16, tag="s2")
            nc.vector.tensor_mul(s2_sb[:], s_sb[:], s_sb[:])

            # ---- d = 1/sqrt(sum_kc K^2 s^2 + eps) as [1, F] ----
            k2_sb = pre.tile([128, KW * F], dt.bfloat16, tag="k2")
            nc.vector.tensor_mul(k2_sb[:], ker_flat[:], ker_flat[:])
            dpre = pre_psum.tile([1, F], dt.float32, tag="pp")
            for k in range(KW):
                nc.tensor.matmul(
                    dpre[:],
                    lhsT=s2_sb[0:64, :],
                    rhs=k2_sb[0:64, k * F : (k + 1) * F],
                    start=(k == 0),
                    stop=(k == KW - 1),
                )
            # rsqrt(v) = exp(-0.5 * ln(v)) — keeps ACT funcs within one LUT set
            lnv = pre.tile([1, F], dt.float32, tag="lnv")
            nc.scalar.activation(
                lnv[:], dpre[:], mybir.ActivationFunctionType.Ln, bias=eps_sb[0:1, :]
            )
            d_sb = pre.tile([1, F], dt.float32, tag="d")
            nc.scalar.activation(
                d_sb[:], lnv[:], mybir.ActivationFunctionType.Exp, scale=-0.5
            )
            d3_sb = pre.tile([1, KW * F], dt.bfloat16, tag="d3")
            for k in range(KW):
                nc.vector.tensor_copy(d3_sb[:, k * F : (k + 1) * F], d_sb[:])

            # ---- w''[(h,c),(k,f)] = K[k,c,f] * d[f] * s[c], bf16 ----
            ones = pre.tile([1, 64], dt.bfloat16, tag="ones")
            nc.vector.memset(ones[:], 1.0)
            dmat = pre_psum.tile([128, KW * F], dt.float32, tag="pp")
            for h in range(2):
                nc.tensor.matmul(
                    dmat[h * 64 : (h + 1) * 64, :],
                    lhsT=ones[:],
                    rhs=d3_sb[:],
                    start=True,
                    stop=True,
                )
            wtmp = pre.tile([128, KW * F], dt.float32, tag="wtmp")
            nc.vector.tensor_mul(wtmp[:], ker_flat[:], dmat[:])
            wfin = pre.tile([128, KW * F], dt.bfloat16, tag="wfin")
            nc.vector.tensor_scalar_mul(wfin[:], wtmp[:], s_sb[:])

            # ---- main conv loop: 4 concurrent 64x64 PE quadrants per tap,
            # two 512-col windows (W0->bank X normal layout, W1->bank Y with
            # partition halves swapped; the host unswizzles odd windows) ----
            yc = [
                yout_pool.tile(
                    [128, CHW], dt.bfloat16, name=f"yout{c}", tag=f"yout{c}"
                )
                for c in range(NCHUNK)
            ]
            PPC = NGRP // 2 // NCHUNK  # pairs per chunk
            for gp in range(NGRP // 2):
                c = gp // PPC
                j0 = (gp % PPC) * 2
                b0, b1 = j0 * 512, (j0 + 1) * 512
                psX = conv_psum.tile([128, 512], dt.float32, name="psX", tag="convps")
                psY = conv_psum.tile([128, 512], dt.float32, name="psY", tag="convps")
                x = xc[c]
                for k in range(KW):
                    st, sp = (k == 0), (k == KW - 1)
                    wA = wfin[0:64, k * F : (k + 1) * F]
                    wB = wfin[64:128, k * F : (k + 1) * F]
                    nc.tensor.matmul(
                        psX[0:64, :], lhsT=wA, rhs=x[0:64, b0 + k : b0 + k + 512],
                        start=st, stop=sp, skip_group_check=True,
                    )
                    nc.tensor.matmul(
                        psX[64:128, :], lhsT=wB, rhs=x[64:128, b0 + k : b0 + k + 512],
                        start=st, stop=sp, skip_group_check=True,
                    )
                    nc.tensor.matmul(
                        psY[64:128, :], lhsT=wA, rhs=x[0:64, b1 + k : b1 + k + 512],
                        start=st, stop=sp, skip_group_check=True,
                    )
                    nc.tensor.matmul(
                        psY[0:64, :], lhsT=wB, rhs=x[64:128, b1 + k : b1 + k + 512],
                        start=st, stop=sp, skip_group_check=True,
                    )
                for w, ps in ((0, psX), (1, psY)):
                    dst = yc[c][:, b0 + w * 512 : b0 + w * 512 + 512]
                    if (2 * gp + w) % 2 == 0:
                        nc.vector.tensor_copy(dst, ps[:])
                    else:
                        nc.scalar.copy(dst, ps[:])
                if gp % 2 == 1:
                    p0 = (j0 - 2) * 512
                    if gp == NGRP // 2 - 1:
                        # final piece split so the trailing transfer is short
                        for q in (0, 1024):
                            nc.sync.dma_start(
                                out=yout[:, c * CHW + p0 + q : c * CHW + p0 + q + 1024],
                                in_=yc[c][:, p0 + q : p0 + q + 1024],
                            )
                    else:
                        nc.sync.dma_start(
                            out=yout[:, c * CHW + p0 : c * CHW + p0 + 2048],
                            in_=yc[c][:, p0 : p0 + 2048],
                        )

    nc.compile()
    return nc


def _get_nc():
    if "nc" not in _cached:
        _cached["nc"] = _build()
    return _cached["nc"]


def pack_params(ltnt_b, kernel, Wd, bd):
    """[128, PARW] f32: wd dup'd per half (2 x [128,128]), lt, bd dup, ker."""
    par = np.empty((128, PARW), dtype=BF16)
    for a in range(2):
        par[:, a * 128 : a * 128 + 64] = Wd[a * 128 : (a + 1) * 128, :]
        par[:, a * 128 + 64 : (a + 1) * 128] = Wd[a * 128 : (a + 1) * 128, :]
    par[:, 256] = ltnt_b[0:128]
    par[:, 257] = ltnt_b[128:256]
    par[:, 258] = np.tile(bd, 2)
    # ker block: par[p, 259 + k*64 + f] = kernel[k, p % 64, f]
    kblk = kernel.transpose(1, 0, 2).reshape(64, KW * F)  # [c, (k,f)]
    par[:, 259:] = np.tile(kblk, (2, 1))
    return par


def make_xin(data_b):
    xt = data_b.reshape(2, H, C).transpose(0, 2, 1)  # [2, C, H]
    xin = np.zeros((128, H + 2), dtype=BF16)
    xin[:, 1 : H + 1] = xt.reshape(128, H).astype(BF16)
    xin[64:128, 0] = xt[0, :, -1].astype(BF16)  # x[H-1] left halo of half 1
    xin[0:64, H + 1] = xt[1, :, 0].astype(BF16)  # x[H] right halo of half 0
    return xin


def kernel(data, ltnt, kernel, Wd, bd):
    from concourse import bass_utils

    nc = _get_nc()

    data = np.asarray(data, dtype=np.float32)
    ltnt = np.asarray(ltnt, dtype=np.float32)
    kf = np.asarray(kernel, dtype=np.float32)
    wdf = np.asarray(Wd, dtype=np.float32)
    bdf = np.asarray(bd, dtype=np.float32)

    in_maps = [
        {"xin": make_xin(data[b]), "par": pack_params(ltnt[b], kf, wdf, bdf)}
        for b in range(B)
    ]

    try:
        res = bass_utils.run_bass_kernel_spmd(nc, in_maps, core_ids=list(range(B)))
    except Exception:
        # transient NRT_EXEC_UNIT_UNRECOVERABLE seen when the device was left
        # wedged by a prior process; one retry after a pause clears it
        import time

        time.sleep(15)
        res = bass_utils.run_bass_kernel_spmd(nc, in_maps, core_ids=list(range(B)))

    out = np.empty((B, L, C), dtype=np.float32)
    even = (np.arange(NGRP) % 2 == 0)[None, :, None]
    for b in range(B):
        yo = np.asarray(res.results[b]["yout"]).astype(np.float32)  # [128, H]
        yr = yo.reshape(2, F, NGRP, 512)  # [rowhalf, f, window, l]
        h0 = np.where(even, yr[0], yr[1])  # odd windows come halves-swapped
        h1 = np.where(even, yr[1], yr[0])
        out[b, :H] = h0.transpose(1, 2, 0).reshape(H, F)
        out[b, H:] = h1.transpose(1, 2, 0).reshape(H, F)
    return out
